# revision 46
# baseline (speedup 1.0000x reference)
"""Trainium2 Bass kernel for nn_ConvGRU: 2-layer GRU, B=32, T=512, D=H=512.

Layer-pipelined across 8 NeuronCores: cores 0-3 run layer 1 on batch
lanes of 8 sequences, cores 4-7 run layer 2 on the same lanes, halving
each core's sequential step count (640 pipeline step-slots instead of
1024).  h1 chunks (64 steps) flow core i -> core i+4 via
ReduceScatter(add) pair collectives: send slot0 = zeros, slot1 =
chunk * smask (smask = 1 only on layer-1 cores), so rank-0 cores
receive zeros and rank-1 cores receive exactly the h1 chunk.  Every
core computes proj_input = x_staged + recv -- layer-1 cores get x + 0,
layer-2 cores get 0 + h1 -- keeping the SPMD program fully symmetric
(no branches, no register-indexed APs).

The 10 pipeline stages are split across 3 sequential TileContexts
(4+4+2) because the Tile scheduler exhausts its ~106 kernel semaphores
at >= 5 hardware loops per context; h state and comm buffers persist
across contexts in internal DRAM.

Per-step recurrence schedule (the per-slot critical path):
- Recurrent weights are e4m3 fp8 scaled by 256 (FWL loads fp8 stationary
  4 elements/cycle vs 2 for fp16, halving the dominant LDWEIGHTS cost of
  48 weight tiles per step); the x-side pregates and biases are
  pre-scaled by 256 on the host and the gate activations apply
  scale=1/256, so PSUM arithmetic is exact in the scaled domain.
- One fused DVE copy preloads the next step's x-pregates into an
  alternate [128, 3G] PSUM tile (bufs=2 pool) while the current step
  computes; matmuls accumulate onto it with start=False.
- PE order per step: r-block (m-major, 16 MMs; sigmoid(r) fires as soon
  as it completes), u-block (runs under sigmoid/rh), o-block on r*h.
- Update algebra h' = (h - u*h) + u*o: u*h and (h - u*h) are computed
  during the o-matmuls, so only u*o and one add follow tanh; tanh and
  the final two ops are split into halves so the next step's first
  matmuls (which read only the first half of h, tracked by subtile
  deps) start before the full update retires.
- The per-step h snapshot into the staging buffer runs on the otherwise
  idle gpsimd engine.
"""

import os
import sys

import numpy as np
import ml_dtypes

sys.path.insert(0, "/opt/trn_rl_repo")
os.environ.setdefault("MYCRO_LOCAL_CACHE", "1")

import concourse.bass as bass  # noqa: E402
import concourse.tile as tile  # noqa: E402
from concourse import mybir  # noqa: E402
from concourse.bass import ds, make_scalar_value, RegisterHandles  # noqa: E402
from concourse.bass_utils import run_bass_kernel_spmd  # noqa: E402
from concourse.vector_clock import ScopedClock  # noqa: E402

FP16 = mybir.dt.float16
FP32 = mybir.dt.float32
FP8 = mybir.dt.float8e4
AF = mybir.ActivationFunctionType
ALU = mybir.AluOpType

N_CORES = 8
LANES = 4
B_FULL, T, D, H, L = 32, 512, 512, 512, 2
B = B_FULL // LANES            # 8 sequences per lane
KC = H // 128                  # 4 contraction chunks
MR = KC                        # psum out chunks per gate
NCH = 3 * MR                   # 12
C = 8                          # time chunks
S = T // C                     # 64 steps per chunk
SB = S * B                     # 512 columns per chunk
NS = C + 2                     # pipeline stages (lag 2)
SCL = 256.0                    # weight pre-scale (h/256 streamed)
U = 64                         # recurrence unroll inside For_i
G = MR * B
GROUPS = [[i, i + LANES] for i in range(LANES)]
_DRAIN_CHUNK = 1
STAGE_GROUPS = [(0, 1, 2, 3), (4, 5, 6, 7), (8, 9)]
STRAIGHT = True                # emit stages without the 1-iteration For_i
INTERLEAVE_PROJ = False        # measured a wash vs the burst; keep burst


class _PatchedTileContext(tile.TileContext):
    """TileContext whose exit drain carries at most _DRAIN_CHUNK sem waits
    per drain instruction (walrus v3 codegen rejects multi-wait drains)."""

    def _drain_and_barrier(self, tick_clock, wait_clock):
        nc = self.nc
        drain_inst = nc.sync.drain()
        wait_clock.add_sem_waits(
            drain_inst.ins, ScopedClock({None: tick_clock.global_clock})
        )
        si = drain_inst.ins.sync_info
        waits = list(si.on_wait) if si is not None else []
        ups = list(si.on_update) if si is not None else []
        if len(waits) > _DRAIN_CHUNK:
            drain_inst.ins.sync_info = mybir.SyncInfo(
                on_wait=waits[:_DRAIN_CHUNK], on_update=[])
            rest = waits[_DRAIN_CHUNK:]
            for i in range(0, len(rest), _DRAIN_CHUNK):
                d2 = nc.sync.drain()
                d2.ins.sync_info = mybir.SyncInfo(
                    on_wait=rest[i:i + _DRAIN_CHUNK],
                    on_update=ups if i + _DRAIN_CHUNK >= len(rest) else [])
        nc.all_engine_barrier()
        popped = nc._tile_sem_poison_stack.pop()
        assert popped is self._sem_poison
        nc.clear_and_free_semaphores(list(self.sems.allocated().values()))
        nc.all_engine_barrier()


_MAX_WAITS = 1


def _split_sync_waits(nc, maxw=_MAX_WAITS):
    """walrus v2/v3 codegen rejects instructions carrying several sync
    waits ("Too many sync wait commands"); split them into preceding
    single-wait NoOps on the same engine."""
    n_new = 0
    for f in nc.m.functions:
        for bb in f.blocks:
            insts = list(bb.instructions)
            out = []
            changed = False
            for inst in insts:
                si = inst.sync_info
                waits = list(si.on_wait) if si is not None and si.on_wait else []
                if len(waits) > maxw:
                    ups = list(si.on_update) if si.on_update else []
                    k = len(waits)
                    for i in range(0, k - maxw, maxw):
                        nop = mybir.InstNoOp(
                            name=f"{inst.name}-wsplit{i}", engine=inst.engine,
                            sync_info=mybir.SyncInfo(
                                on_wait=waits[i:i + maxw], on_update=[]))
                        out.append(nop)
                        n_new += 1
                    inst.sync_info = mybir.SyncInfo(
                        on_wait=waits[k - maxw:], on_update=ups)
                    changed = True
                out.append(inst)
            if changed:
                bb.instructions = out
    return n_new


def build_gru2(step_count=None, fp8=True, unroll=None):
    if step_count is None:
        step_count = S
    WDT = FP8 if fp8 else FP16
    nc = bass.Bass(num_devices=N_CORES)

    whru_d = nc.declare_dram_parameter("whru", [KC, 128, 2 * H], WDT, isOutput=False)
    who_d = nc.declare_dram_parameter("who", [KC, 128, H], WDT, isOutput=False)
    wxru_d = nc.declare_dram_parameter("wxru", [KC, 128, 2 * H], FP16, isOutput=False)
    wxo_d = nc.declare_dram_parameter("wxo", [KC, 128, H], FP16, isOutput=False)
    bias_d = nc.declare_dram_parameter("bias", [128, NCH], FP32, isOutput=False)
    srcx_d = nc.declare_dram_parameter("srcx", [C, KC, 128, SB], FP16, isOutput=False)
    smask_d = nc.declare_dram_parameter("smask", [128, KC * SB], FP16, isOutput=False)
    hmask_d = nc.declare_dram_parameter("hmask", [128, KC * B], FP16, isOutput=False)
    out_d = nc.declare_dram_parameter("out", [2, H, B], FP16, isOutput=True)

    send_t = [nc.dram_tensor(f"sendbuf{p}", [2, KC, 128, SB], FP16, kind="Internal")
              for p in range(2)]
    recv_t = [nc.dram_tensor(f"recvbuf{p}", [KC, 128, SB], FP16, kind="Internal")
              for p in range(2)]
    hstate_d = nc.dram_tensor("hstate", [128, 2 * KC * B], FP16, kind="Internal")

    uu = U if unroll is None else unroll
    for gi, group in enumerate(STAGE_GROUPS):
        _build_group(nc, tuple(group), gi, step_count, WDT, uu,
                     whru_d, who_d, wxru_d, wxo_d, bias_d, srcx_d, smask_d,
                     hmask_d, out_d, send_t, recv_t, hstate_d)

    _split_sync_waits(nc)
    return nc


def _build_group(nc, group, gi, step_count, WDT, U,
                 whru_d, who_d, wxru_d, wxo_d, bias_d, srcx_d, smask_d,
                 hmask_d, out_d, send_t, recv_t, hstate_d):
    first_ctx = gi == 0
    last_ctx = gi == len(STAGE_GROUPS) - 1
    with _PatchedTileContext(nc) as tc:
        with (
            tc.tile_pool(name="weights", bufs=1) as wpool,
            tc.tile_pool(name="state", bufs=1) as stpool,
            tc.tile_pool(name="xs", bufs=2) as xspool,
            tc.tile_pool(name="rv", bufs=2) as rvpool,
            tc.tile_pool(name="xin", bufs=2) as xinpool,
            tc.tile_pool(name="pall", bufs=2) as papool,
            tc.tile_pool(name="hsb", bufs=2) as hbpool,
            tc.tile_pool(name="snd", bufs=2) as snpool,
            tc.tile_pool(name="psum_g", bufs=2, space="PSUM") as pg_pool,
            tc.tile_pool(name="psum_r", bufs=2, space="PSUM") as pr_pool,
        ):
            # ---- weights / constants ----
            whru_s = wpool.tile([128, KC * 2 * H], WDT, tag="whru")
            who_s = wpool.tile([128, KC * H], WDT, tag="who")
            wxru_s = wpool.tile([128, KC * 2 * H], FP16, tag="wxru")
            wxo_s = wpool.tile([128, KC * H], FP16, tag="wxo")
            bias_s = wpool.tile([128, NCH], FP32, tag="bias")
            for k in range(KC):
                nc.sync.dma_start(whru_s[:, k * 2 * H:(k + 1) * 2 * H], whru_d[k])
                nc.sync.dma_start(who_s[:, k * H:(k + 1) * H], who_d[k])
                nc.sync.dma_start(wxru_s[:, k * 2 * H:(k + 1) * 2 * H], wxru_d[k])
                nc.sync.dma_start(wxo_s[:, k * H:(k + 1) * H], wxo_d[k])
            nc.sync.dma_start(bias_s[:], bias_d[:])
            smask_s = hmask_s = None
            if not last_ctx:
                smask_s = wpool.tile([128, KC * SB], FP16, tag="smask")
                nc.sync.dma_start(smask_s[:], smask_d[:])
            if first_ctx:
                hmask_s = wpool.tile([128, KC * B], FP16, tag="hmask")
                nc.sync.dma_start(hmask_s[:], hmask_d[:])

            # ---- state ----
            hT = stpool.tile([128, KC * B], FP16, tag="hT")
            if first_ctx:
                zt = stpool.tile([128, SB], FP16, tag="zt")
                nc.vector.memset(zt[:], 0.0)
                for p in range(2):
                    for k in range(KC):
                        nc.gpsimd.dma_start(send_t[p][0, k], zt[:])
                nc.vector.memset(hT[:], 0.0)
            else:
                nc.sync.dma_start(hT[:], hstate_d[:, 0:KC * B])
            h3 = hT[:].rearrange("p (c b) -> p c b", c=KC)

            gates = stpool.tile([128, 7 * G], FP16, tag="gates")
            rT = gates[:, 0:G]
            uT = gates[:, G:2 * G]
            oT = gates[:, 2 * G:3 * G]
            rhT = gates[:, 3 * G:4 * G]
            uhT = gates[:, 4 * G:5 * G]
            eT = gates[:, 5 * G:6 * G]
            fT = gates[:, 6 * G:7 * G]

            # ---- helpers ----
            def stage_xin_dma(s):
                """Issue the input DMAs for stage s; the combining add is
                deferred (see stage_xin)."""
                chunk = min(s, C - 1)
                xs = xspool.tile([128, KC * SB], FP16, tag="xs")
                for k in range(KC):
                    nc.sync.dma_start(xs[:, k * SB:(k + 1) * SB], srcx_d[chunk, k])
                if s < 2:
                    return xs, None
                rv = rvpool.tile([128, KC * SB], FP16, tag="rv")
                for k in range(KC):
                    nc.sync.dma_start(rv[:, k * SB:(k + 1) * SB], recv_t[s % 2][k])
                return xs, rv

            def stage_xin_add(xs, rv):
                if rv is None:
                    return xs
                xin = xinpool.tile([128, KC * SB], FP16, tag="xin")
                nc.vector.tensor_add(xin[:], xs[:], rv[:])
                return xin

            def stage_xin(s):
                xs, rv = stage_xin_dma(s)
                return stage_xin_add(xs, rv)

            def proj_mm(xin, pall, m, k, ps):
                """One projection matmul (m-chunk m, contraction chunk k);
                emits the bias/ACT when the accumulation group completes.
                SB<=512 assumed (single free-dim group)."""
                if m < 2 * MR:
                    lhsT = wxru_s[:, k * 2 * H + m * 128:
                                  k * 2 * H + (m + 1) * 128]
                else:
                    mo = m - 2 * MR
                    lhsT = wxo_s[:, k * H + mo * 128:
                                 k * H + (mo + 1) * 128]
                nc.tensor.matmul(ps[:, :SB], lhsT,
                                 xin[:, k * SB:(k + 1) * SB],
                                 start=(k == 0), stop=(k == KC - 1))
                if k == KC - 1:
                    nc.scalar.activation(
                        pall[:, m * SB:(m + 1) * SB], ps[:, :SB],
                        AF.Identity, bias=bias_s[:, m:m + 1])

            def proj_chunk(xin, pall, m):
                ps = pg_pool.tile([128, 512], FP32, tag="pg")
                for k in range(KC):
                    proj_mm(xin, pall, m, k, ps)

            def proj(xin, pall):
                for m in range(NCH):
                    proj_chunk(xin, pall, m)

            ascl = 1.0 / SCL if WDT == FP8 else 1.0
            HB = KC * B // 2          # half of the h free-dim (k-chunks 0-1)

            def preload(stage3, uu):
                """One fused DVE copy of step uu's x-pregates into a fresh
                [128, 3G] PSUM tile (bufs=2 pool => runs under step uu-1)."""
                pst = pr_pool.tile([128, NCH * B], FP32, tag="pst")
                pst3 = pst[:].rearrange("p (c b) -> p c b", c=NCH)
                nc.vector.tensor_copy(pst3, stage3[:, :, uu * B:(uu + 1) * B])
                return pst

            def recurrence_step(stage3, hstage3, uu, pst, pst_next_uu):
                s0 = uu * B
                psr = pst[:, 0:G]
                psu = pst[:, G:2 * G]
                pso = pst[:, 2 * G:3 * G]
                # m-major r block first so sigmoid(r) fires after 16 MMs;
                # u block follows (its sigmoid is only needed late).  The
                # first MM of each m-block reads h[k0] only, so the next
                # step still starts on the early-retired first h half
                # (subtile deps).
                for gm in range(2 * MR):
                    for k in range(KC):
                        nc.tensor.matmul(
                            pst[:, gm * B:(gm + 1) * B],
                            whru_s[:, k * 2 * H + gm * 128:
                                   k * 2 * H + (gm + 1) * 128],
                            hT[:, k * B:(k + 1) * B],
                            start=False, stop=(k == KC - 1),
                            skip_group_check=True)
                    if gm == MR - 1:
                        nc.scalar.activation(rT, psr, AF.Sigmoid, scale=ascl)
                nc.scalar.activation(uT, psu, AF.Sigmoid, scale=ascl)
                nc.vector.tensor_mul(rhT, rT, hT[:])
                nc.vector.tensor_mul(uhT, uT, hT[:])
                nc.vector.tensor_sub(eT, hT[:], uhT)   # e = (1-u)*h
                for m in range(MR):
                    for k in range(KC):
                        nc.tensor.matmul(
                            pso[:, m * B:(m + 1) * B],
                            who_s[:, k * H + m * 128:k * H + (m + 1) * 128],
                            rhT[:, k * B:(k + 1) * B],
                            start=False, stop=(k == KC - 1),
                            skip_group_check=True)
                # preload the next step's pregates while o/tanh run
                pst_next = None
                if pst_next_uu is not None:
                    pst_next = preload(stage3, pst_next_uu)
                # tanh + h-update in halves: h = e + u*o, first half retires
                # early so the next step's k=0 sweep can begin
                for hh in (0, 1):
                    lo, hi = hh * HB, (hh + 1) * HB
                    nc.scalar.activation(oT[:, lo:hi], pso[:, lo:hi],
                                         AF.Tanh, scale=ascl)
                    nc.vector.tensor_mul(fT[:, lo:hi], uT[:, lo:hi],
                                         oT[:, lo:hi])
                    nc.vector.tensor_add(hT[:, lo:hi], eT[:, lo:hi],
                                         fT[:, lo:hi])
                nc.gpsimd.tensor_copy(hstage3[:, :, s0:s0 + B], h3)
                return pst_next

            # ---- group prologue: first stage's xin + proj ----
            palls = {}
            xin0 = stage_xin(group[0])
            pall0 = papool.tile([128, NCH * SB], FP16, tag="pall")
            proj(xin0, pall0)
            palls[group[0]] = pall0

            # ---- stages ----
            for s in group:
                pall = palls.pop(s)
                pall3 = pall[:].rearrange("p (c t) -> p c t", c=NCH)
                hsb = hbpool.tile([128, KC * SB], FP16, tag="hsb")
                hsb3 = hsb[:].rearrange("p (c t) -> p c t", c=KC)

                # single-iteration (or no) hardware loop: all offsets static,
                # the recurrence reads pall and writes hsb directly (no bulk
                # staging copies).  The next stage's projection is interleaved
                # into the second half of the steps (one m-chunk per ~2 steps)
                # so its PE work hides under recurrence chain slack; its input
                # add fires mid-stage, safely after the lag-2 collective.
                nu = min(U, step_count)
                inter = INTERLEAVE_PROJ and STRAIGHT and s + 1 in group and nu >= 56
                if inter:
                    pall_n = papool.tile([128, NCH * SB], FP16, tag="pall")
                    xs_n, rv_n = stage_xin_dma(s + 1)
                    xin_n = None
                    ps_n = None
                if STRAIGHT:
                    pst = preload(pall3, 0)
                    for uu in range(nu):
                        pst = recurrence_step(
                            pall3, hsb3, uu, pst,
                            uu + 1 if uu + 1 < nu else None)
                        if inter:
                            # one 512-col projection MM per step (~270ns)
                            # fits the per-step PE gap during tanh/update
                            if uu == 14:
                                xin_n = stage_xin_add(xs_n, rv_n)
                            elif 16 <= uu < 16 + KC * NCH:
                                mi = uu - 16
                                m, k = mi // KC, mi % KC
                                if k == 0:
                                    ps_n = pg_pool.tile([128, 512], FP32,
                                                        tag="pg")
                                proj_mm(xin_n, pall_n, m, k, ps_n)
                    if inter:
                        palls[s + 1] = pall_n
                else:
                    with tc.For_i(0, step_count, nu) as i:
                        pst = preload(pall3, 0)
                        for uu in range(nu):
                            pst = recurrence_step(
                                pall3, hsb3, uu, pst,
                                uu + 1 if uu + 1 < nu else None)
                    if hasattr(i, "val"):
                        for r in i.val:
                            nc.free_register(r)

                # next stage's staging + projection (burst fallback when not
                # interleaved above)
                if s + 1 in group and not inter:
                    pall_n = papool.tile([128, NCH * SB], FP16, tag="pall")
                    xin_n = stage_xin(s + 1)
                    proj(xin_n, pall_n)
                    palls[s + 1] = pall_n

                if s < 2:
                    nc.vector.tensor_mul(hT[:], hT[:], hmask_s[:])
                if s == C - 1:
                    out0 = out_d[0].rearrange("(c p) b -> p c b", p=128)
                    nc.sync.dma_start(out0, h3)
                if s == NS - 1:
                    out1 = out_d[1].rearrange("(c p) b -> p c b", p=128)
                    nc.sync.dma_start(out1, h3)

                if s < C:
                    snd = snpool.tile([128, KC * SB], FP16, tag="snd")
                    nc.vector.tensor_mul(snd[:], hsb[:], smask_s[:])
                    for k in range(KC):
                        nc.gpsimd.dma_start(send_t[s % 2][1, k],
                                            snd[:, k * SB:(k + 1) * SB])
                    nc.gpsimd.collective_compute(
                        "ReduceScatter", ALU.add,
                        replica_groups=GROUPS,
                        ins=[send_t[s % 2][:].opt()],
                        outs=[recv_t[s % 2][:].opt()],
                    )

            if not last_ctx:
                nc.sync.dma_start(hstate_d[:, 0:KC * B], hT[:])


def prep_in_maps(x, Wr, br, Wu, bu, Wo, bo, fp8=True):
    x = np.asarray(x, np.float32)
    Wr, Wu, Wo = (np.asarray(a, np.float32) for a in (Wr, Wu, Wo))
    br, bu, bo = (np.asarray(a, np.float32) for a in (br, bu, bo))

    def quant(w):
        if fp8:
            return (w * SCL).astype(ml_dtypes.float8_e4m3)
        return w.astype(np.float16)

    # With fp8 recurrent weights (pre-scaled by SCL), the x-side weights and
    # biases are also pre-scaled by SCL so PSUM holds SCL*(pre-activation)
    # uniformly; the gate activations then apply scale=1/SCL.
    xscl = SCL if fp8 else 1.0

    layer_weights = []
    for l in range(L):
        w_ru_h = np.concatenate([Wr[l][:, D:], Wu[l][:, D:]], 0)   # (2H, H)
        w_ru_x = np.concatenate([Wr[l][:, :D], Wu[l][:, :D]], 0)
        whru = quant(np.ascontiguousarray(w_ru_h.T.reshape(KC, 128, 2 * H)))
        wxru = (w_ru_x.T.reshape(KC, 128, 2 * H) * xscl).astype(np.float16)
        who = quant(np.ascontiguousarray(Wo[l][:, D:].T.reshape(KC, 128, H)))
        wxo = (Wo[l][:, :D].T.reshape(KC, 128, H) * xscl).astype(np.float16)
        bb = np.concatenate([br[l], bu[l], bo[l]], 0)              # (3H,)
        bias = (np.ascontiguousarray(bb.reshape(NCH, 128).T) * xscl
                ).astype(np.float32)
        layer_weights.append(dict(whru=whru, wxru=wxru, who=who, wxo=wxo,
                                  bias=bias))

    in_maps = []
    for c in range(N_CORES):
        lane, layer = c % LANES, c // LANES
        m = dict(layer_weights[layer])
        if layer == 0:
            xs = x[lane * B:(lane + 1) * B]                        # (B, T, D)
            xt = np.ascontiguousarray(xs.transpose(2, 1, 0))       # (D, T, B)
            srcx = np.ascontiguousarray(
                xt.reshape(KC, 128, C, S * B).transpose(2, 0, 1, 3))
            m["srcx"] = srcx.astype(np.float16)
        else:
            m["srcx"] = np.zeros((C, KC, 128, SB), np.float16)
        mv = 1.0 if layer == 0 else 0.0
        m["smask"] = np.full((128, KC * SB), mv, np.float16)
        m["hmask"] = np.full((128, KC * B), mv, np.float16)
        in_maps.append(m)
    return in_maps


def gather_output(results):
    out = np.zeros((L, B_FULL, H), np.float32)
    for lane in range(LANES):
        o0 = np.asarray(results[lane]["out"], np.float32)          # (2, H, B)
        o1 = np.asarray(results[lane + LANES]["out"], np.float32)
        out[0, lane * B:(lane + 1) * B] = o0[0].T
        out[1, lane * B:(lane + 1) * B] = o1[1].T
    return out


def kernel2(x, Wr, br, Wu, bu, Wo, bo, fp8=True):
    nc = build_gru2(fp8=fp8)
    in_maps = prep_in_maps(x, Wr, br, Wu, bu, Wo, bo, fp8=fp8)
    res = run_bass_kernel_spmd(nc, in_maps, list(range(N_CORES)))
    return gather_output(res.results)




_NC_CACHE = {}

# Default precision for kernel(): fp8 recurrent weights (scaled by SCL with
# the activation-side 1/SCL compensation). Measured rel-err 1.30e-2 vs the
# fp32 reference (threshold 2e-2); fp16 fallback measured 1.25e-3.
FP8_DEFAULT = True


def _get_nc():
    key = (C, S, U, FP8_DEFAULT)
    if key not in _NC_CACHE:
        _NC_CACHE[key] = build_gru2(fp8=FP8_DEFAULT)
    return _NC_CACHE[key]


# ---- timing support (used by test.py only; harness never calls these) ----
TIMING_STEPS_PAIR = (16, S)
TIMING_SL_DELTA = NS * (S - 16)
REAL_STEP_LAYERS = NS * S


def build_for_timing(steps):
    return build_gru2(step_count=steps, fp8=FP8_DEFAULT,
                      unroll=min(U, steps))


def kernel(x, Wr, br, Wu, bu, Wo, bo):
    nc = _get_nc()
    in_maps = prep_in_maps(x, Wr, br, Wu, bu, Wo, bo, fp8=FP8_DEFAULT)
    res = run_bass_kernel_spmd(nc, in_maps, list(range(N_CORES)))
    return gather_output(res.results)



# revision 49
# speedup vs baseline: 1.1952x; 1.1952x over previous
"""Trainium2 Bass kernel for nn_ConvGRU: 2-layer GRU, B=32, T=512, D=H=512.

Layer-pipelined across 8 NeuronCores: cores 0-3 run layer 1 on batch
lanes of 8 sequences, cores 4-7 run layer 2 on the same lanes, halving
each core's sequential step count (576 pipeline step-slots with C=16
32-step chunks instead of 1024).  h1 chunks flow core i -> core i+4
via ReduceScatter(add) pair collectives: send slot0 = zeros, slot1 =
chunk * smask (smask = 1 only on layer-1 cores), so rank-0 cores
receive zeros and rank-1 cores receive exactly the h1 chunk.  Every
core computes proj_input = x_staged + recv -- layer-1 cores get x + 0,
layer-2 cores get 0 + h1 -- keeping the SPMD program fully symmetric
(no branches).

The 18 pipeline stages are split across 5 sequential TileContexts (4
stages each); h state and comm buffers persist across contexts in
internal DRAM.  Stages are emitted straight-line (no hardware loops,
no registers, no staging copies): the recurrence reads the projected
pregates (pall) and writes the h history (hsb) at static offsets.

Per-step recurrence schedule (the per-slot critical path):
- Recurrent weights are e4m3 fp8 scaled by 256 (FWL loads fp8 stationary
  4 elements/cycle vs 2 for fp16, halving the dominant LDWEIGHTS cost of
  48 weight tiles per step); the x-side pregates and biases are
  pre-scaled by 256 on the host and the gate activations apply
  scale=1/256, so PSUM arithmetic is exact in the scaled domain.
- One fused DVE copy preloads the next step's x-pregates into an
  alternate [128, 3G] PSUM tile (bufs=2 pool) while the current step
  computes; matmuls accumulate onto it with start=False.
- PE order per step: r-block (m-major, 16 MMs; sigmoid(r) fires as soon
  as it completes), u-block (runs under sigmoid/rh), o-block on r*h.
- Update algebra h' = (h - u*h) + u*o: u*h and (h - u*h) are computed
  during the o-matmuls, so only u*o and one add follow tanh; tanh and
  the final two ops are split into halves so the next step's first
  matmuls (which read only the first half of h, tracked by subtile
  deps) start before the full update retires.
- The per-step h snapshot into the staging buffer runs on the otherwise
  idle gpsimd engine.
"""

import os
import sys

import numpy as np
import ml_dtypes

sys.path.insert(0, "/opt/trn_rl_repo")
os.environ.setdefault("MYCRO_LOCAL_CACHE", "1")

import concourse.bass as bass  # noqa: E402
import concourse.tile as tile  # noqa: E402
from concourse import mybir  # noqa: E402
from concourse.bass import ds, make_scalar_value, RegisterHandles  # noqa: E402
from concourse.bass_utils import run_bass_kernel_spmd  # noqa: E402
from concourse.vector_clock import ScopedClock  # noqa: E402

FP16 = mybir.dt.float16
FP32 = mybir.dt.float32
FP8 = mybir.dt.float8e4
AF = mybir.ActivationFunctionType
ALU = mybir.AluOpType

N_CORES = 8
LANES = 4
B_FULL, T, D, H, L = 32, 512, 512, 512, 2
B = B_FULL // LANES            # 8 sequences per lane
KC = H // 128                  # 4 contraction chunks
MR = KC                        # psum out chunks per gate
NCH = 3 * MR                   # 12
C = int(os.environ.get("KERNEL_C", "16"))  # time chunks
S = T // C                     # steps per chunk
SB = S * B                     # columns per chunk
NS = C + 2                     # pipeline stages (lag 2)
SCL = 256.0                    # weight pre-scale (h/256 streamed)
U = 64                         # recurrence unroll (straight-line: >= S)
G = MR * B
GROUPS = [[i, i + LANES] for i in range(LANES)]
_DRAIN_CHUNK = 1
STAGE_GROUPS = [tuple(range(i, min(i + 4, NS))) for i in range(0, NS, 4)]
STRAIGHT = True                # emit stages without the 1-iteration For_i
INTERLEAVE_PROJ = False        # measured a wash vs the burst; keep burst


class _PatchedTileContext(tile.TileContext):
    """TileContext whose exit drain carries at most _DRAIN_CHUNK sem waits
    per drain instruction (walrus v3 codegen rejects multi-wait drains)."""

    def _drain_and_barrier(self, tick_clock, wait_clock):
        nc = self.nc
        drain_inst = nc.sync.drain()
        wait_clock.add_sem_waits(
            drain_inst.ins, ScopedClock({None: tick_clock.global_clock})
        )
        si = drain_inst.ins.sync_info
        waits = list(si.on_wait) if si is not None else []
        ups = list(si.on_update) if si is not None else []
        if len(waits) > _DRAIN_CHUNK:
            drain_inst.ins.sync_info = mybir.SyncInfo(
                on_wait=waits[:_DRAIN_CHUNK], on_update=[])
            rest = waits[_DRAIN_CHUNK:]
            for i in range(0, len(rest), _DRAIN_CHUNK):
                d2 = nc.sync.drain()
                d2.ins.sync_info = mybir.SyncInfo(
                    on_wait=rest[i:i + _DRAIN_CHUNK],
                    on_update=ups if i + _DRAIN_CHUNK >= len(rest) else [])
        nc.all_engine_barrier()
        popped = nc._tile_sem_poison_stack.pop()
        assert popped is self._sem_poison
        nc.clear_and_free_semaphores(list(self.sems.allocated().values()))
        nc.all_engine_barrier()


_MAX_WAITS = 1


def _split_sync_waits(nc, maxw=_MAX_WAITS):
    """walrus v2/v3 codegen rejects instructions carrying several sync
    waits ("Too many sync wait commands"); split them into preceding
    single-wait NoOps on the same engine."""
    n_new = 0
    for f in nc.m.functions:
        for bb in f.blocks:
            insts = list(bb.instructions)
            out = []
            changed = False
            for inst in insts:
                si = inst.sync_info
                waits = list(si.on_wait) if si is not None and si.on_wait else []
                if len(waits) > maxw:
                    ups = list(si.on_update) if si.on_update else []
                    k = len(waits)
                    for i in range(0, k - maxw, maxw):
                        nop = mybir.InstNoOp(
                            name=f"{inst.name}-wsplit{i}", engine=inst.engine,
                            sync_info=mybir.SyncInfo(
                                on_wait=waits[i:i + maxw], on_update=[]))
                        out.append(nop)
                        n_new += 1
                    inst.sync_info = mybir.SyncInfo(
                        on_wait=waits[k - maxw:], on_update=ups)
                    changed = True
                out.append(inst)
            if changed:
                bb.instructions = out
    return n_new


def build_gru2(step_count=None, fp8=True, unroll=None):
    if step_count is None:
        step_count = S
    WDT = FP8 if fp8 else FP16
    nc = bass.Bass(num_devices=N_CORES)

    whru_d = nc.declare_dram_parameter("whru", [KC, 128, 2 * H], WDT, isOutput=False)
    who_d = nc.declare_dram_parameter("who", [KC, 128, H], WDT, isOutput=False)
    wxru_d = nc.declare_dram_parameter("wxru", [KC, 128, 2 * H], FP16, isOutput=False)
    wxo_d = nc.declare_dram_parameter("wxo", [KC, 128, H], FP16, isOutput=False)
    bias_d = nc.declare_dram_parameter("bias", [128, NCH], FP32, isOutput=False)
    srcx_d = nc.declare_dram_parameter("srcx", [C, KC, 128, SB], FP16, isOutput=False)
    smask_d = nc.declare_dram_parameter("smask", [128, KC * SB], FP16, isOutput=False)
    hmask_d = nc.declare_dram_parameter("hmask", [128, KC * B], FP16, isOutput=False)
    out_d = nc.declare_dram_parameter("out", [2, H, B], FP16, isOutput=True)

    send_t = [nc.dram_tensor(f"sendbuf{p}", [2, KC, 128, SB], FP16, kind="Internal")
              for p in range(2)]
    recv_t = [nc.dram_tensor(f"recvbuf{p}", [KC, 128, SB], FP16, kind="Internal")
              for p in range(2)]
    hstate_d = nc.dram_tensor("hstate", [128, 2 * KC * B], FP16, kind="Internal")

    uu = U if unroll is None else unroll
    for gi, group in enumerate(STAGE_GROUPS):
        _build_group(nc, tuple(group), gi, step_count, WDT, uu,
                     whru_d, who_d, wxru_d, wxo_d, bias_d, srcx_d, smask_d,
                     hmask_d, out_d, send_t, recv_t, hstate_d)

    _split_sync_waits(nc)
    return nc


def _build_group(nc, group, gi, step_count, WDT, U,
                 whru_d, who_d, wxru_d, wxo_d, bias_d, srcx_d, smask_d,
                 hmask_d, out_d, send_t, recv_t, hstate_d):
    first_ctx = gi == 0
    last_ctx = gi == len(STAGE_GROUPS) - 1
    with _PatchedTileContext(nc) as tc:
        with (
            tc.tile_pool(name="weights", bufs=1) as wpool,
            tc.tile_pool(name="state", bufs=1) as stpool,
            tc.tile_pool(name="xs", bufs=2) as xspool,
            tc.tile_pool(name="rv", bufs=2) as rvpool,
            tc.tile_pool(name="xin", bufs=2) as xinpool,
            tc.tile_pool(name="pall", bufs=2) as papool,
            tc.tile_pool(name="hsb", bufs=2) as hbpool,
            tc.tile_pool(name="snd", bufs=2) as snpool,
            tc.tile_pool(name="psum_g", bufs=2, space="PSUM") as pg_pool,
            tc.tile_pool(name="psum_r", bufs=2, space="PSUM") as pr_pool,
        ):
            # ---- weights / constants ----
            whru_s = wpool.tile([128, KC * 2 * H], WDT, tag="whru")
            who_s = wpool.tile([128, KC * H], WDT, tag="who")
            wxru_s = wpool.tile([128, KC * 2 * H], FP16, tag="wxru")
            wxo_s = wpool.tile([128, KC * H], FP16, tag="wxo")
            bias_s = wpool.tile([128, NCH], FP32, tag="bias")
            for k in range(KC):
                nc.sync.dma_start(whru_s[:, k * 2 * H:(k + 1) * 2 * H], whru_d[k])
                nc.sync.dma_start(who_s[:, k * H:(k + 1) * H], who_d[k])
                nc.sync.dma_start(wxru_s[:, k * 2 * H:(k + 1) * 2 * H], wxru_d[k])
                nc.sync.dma_start(wxo_s[:, k * H:(k + 1) * H], wxo_d[k])
            nc.sync.dma_start(bias_s[:], bias_d[:])
            smask_s = hmask_s = None
            if not last_ctx:
                smask_s = wpool.tile([128, KC * SB], FP16, tag="smask")
                nc.sync.dma_start(smask_s[:], smask_d[:])
            if first_ctx:
                hmask_s = wpool.tile([128, KC * B], FP16, tag="hmask")
                nc.sync.dma_start(hmask_s[:], hmask_d[:])

            # ---- state ----
            hT = stpool.tile([128, KC * B], FP16, tag="hT")
            if first_ctx:
                zt = stpool.tile([128, SB], FP16, tag="zt")
                nc.vector.memset(zt[:], 0.0)
                for p in range(2):
                    for k in range(KC):
                        nc.gpsimd.dma_start(send_t[p][0, k], zt[:])
                nc.vector.memset(hT[:], 0.0)
            else:
                nc.sync.dma_start(hT[:], hstate_d[:, 0:KC * B])
            h3 = hT[:].rearrange("p (c b) -> p c b", c=KC)

            gates = stpool.tile([128, 7 * G], FP16, tag="gates")
            rT = gates[:, 0:G]
            uT = gates[:, G:2 * G]
            oT = gates[:, 2 * G:3 * G]
            rhT = gates[:, 3 * G:4 * G]
            uhT = gates[:, 4 * G:5 * G]
            eT = gates[:, 5 * G:6 * G]
            fT = gates[:, 6 * G:7 * G]

            # ---- helpers ----
            def stage_xin_dma(s):
                """Issue the input DMAs for stage s; the combining add is
                deferred (see stage_xin)."""
                chunk = min(s, C - 1)
                xs = xspool.tile([128, KC * SB], FP16, tag="xs")
                for k in range(KC):
                    nc.sync.dma_start(xs[:, k * SB:(k + 1) * SB], srcx_d[chunk, k])
                if s < 2:
                    return xs, None
                rv = rvpool.tile([128, KC * SB], FP16, tag="rv")
                for k in range(KC):
                    nc.sync.dma_start(rv[:, k * SB:(k + 1) * SB], recv_t[s % 2][k])
                return xs, rv

            def stage_xin_add(xs, rv):
                if rv is None:
                    return xs
                xin = xinpool.tile([128, KC * SB], FP16, tag="xin")
                nc.vector.tensor_add(xin[:], xs[:], rv[:])
                return xin

            def stage_xin(s):
                xs, rv = stage_xin_dma(s)
                return stage_xin_add(xs, rv)

            def proj_mm(xin, pall, m, k, ps):
                """One projection matmul (m-chunk m, contraction chunk k);
                emits the bias/ACT when the accumulation group completes.
                SB<=512 assumed (single free-dim group)."""
                if m < 2 * MR:
                    lhsT = wxru_s[:, k * 2 * H + m * 128:
                                  k * 2 * H + (m + 1) * 128]
                else:
                    mo = m - 2 * MR
                    lhsT = wxo_s[:, k * H + mo * 128:
                                 k * H + (mo + 1) * 128]
                nc.tensor.matmul(ps[:, :SB], lhsT,
                                 xin[:, k * SB:(k + 1) * SB],
                                 start=(k == 0), stop=(k == KC - 1))
                if k == KC - 1:
                    nc.scalar.activation(
                        pall[:, m * SB:(m + 1) * SB], ps[:, :SB],
                        AF.Identity, bias=bias_s[:, m:m + 1])

            def proj_chunk(xin, pall, m):
                ps = pg_pool.tile([128, 512], FP32, tag="pg")
                for k in range(KC):
                    proj_mm(xin, pall, m, k, ps)

            def proj(xin, pall):
                for m in range(NCH):
                    proj_chunk(xin, pall, m)

            ascl = 1.0 / SCL if WDT == FP8 else 1.0
            HB = KC * B // 2          # half of the h free-dim (k-chunks 0-1)

            def preload(stage3, uu):
                """One fused DVE copy of step uu's x-pregates into a fresh
                [128, 3G] PSUM tile (bufs=2 pool => runs under step uu-1)."""
                pst = pr_pool.tile([128, NCH * B], FP32, tag="pst")
                pst3 = pst[:].rearrange("p (c b) -> p c b", c=NCH)
                nc.vector.tensor_copy(pst3, stage3[:, :, uu * B:(uu + 1) * B])
                return pst

            def recurrence_step(stage3, hstage3, uu, pst, pst_next_uu):
                s0 = uu * B
                psr = pst[:, 0:G]
                psu = pst[:, G:2 * G]
                pso = pst[:, 2 * G:3 * G]
                # m-major r block first so sigmoid(r) fires after 16 MMs;
                # u block follows (its sigmoid is only needed late).  The
                # first MM of each m-block reads h[k0] only, so the next
                # step still starts on the early-retired first h half
                # (subtile deps).
                for gm in range(2 * MR):
                    for k in range(KC):
                        nc.tensor.matmul(
                            pst[:, gm * B:(gm + 1) * B],
                            whru_s[:, k * 2 * H + gm * 128:
                                   k * 2 * H + (gm + 1) * 128],
                            hT[:, k * B:(k + 1) * B],
                            start=False, stop=(k == KC - 1),
                            skip_group_check=True)
                    if gm == MR - 1:
                        nc.scalar.activation(rT, psr, AF.Sigmoid, scale=ascl)
                nc.scalar.activation(uT, psu, AF.Sigmoid, scale=ascl)
                nc.vector.tensor_mul(rhT, rT, hT[:])
                nc.vector.tensor_mul(uhT, uT, hT[:])
                nc.vector.tensor_sub(eT, hT[:], uhT)   # e = (1-u)*h
                for m in range(MR):
                    for k in range(KC):
                        nc.tensor.matmul(
                            pso[:, m * B:(m + 1) * B],
                            who_s[:, k * H + m * 128:k * H + (m + 1) * 128],
                            rhT[:, k * B:(k + 1) * B],
                            start=False, stop=(k == KC - 1),
                            skip_group_check=True)
                # preload the next step's pregates while o/tanh run
                pst_next = None
                if pst_next_uu is not None:
                    pst_next = preload(stage3, pst_next_uu)
                # tanh + h-update in halves: h = e + u*o, first half retires
                # early so the next step's k=0 sweep can begin
                for hh in (0, 1):
                    lo, hi = hh * HB, (hh + 1) * HB
                    nc.scalar.activation(oT[:, lo:hi], pso[:, lo:hi],
                                         AF.Tanh, scale=ascl)
                    nc.vector.tensor_mul(fT[:, lo:hi], uT[:, lo:hi],
                                         oT[:, lo:hi])
                    nc.vector.tensor_add(hT[:, lo:hi], eT[:, lo:hi],
                                         fT[:, lo:hi])
                nc.gpsimd.tensor_copy(hstage3[:, :, s0:s0 + B], h3)
                return pst_next

            # ---- group prologue: first stage's xin + proj ----
            palls = {}
            xin0 = stage_xin(group[0])
            pall0 = papool.tile([128, NCH * SB], FP16, tag="pall")
            proj(xin0, pall0)
            palls[group[0]] = pall0

            # ---- stages ----
            for s in group:
                pall = palls.pop(s)
                pall3 = pall[:].rearrange("p (c t) -> p c t", c=NCH)
                hsb = hbpool.tile([128, KC * SB], FP16, tag="hsb")
                hsb3 = hsb[:].rearrange("p (c t) -> p c t", c=KC)

                # single-iteration (or no) hardware loop: all offsets static,
                # the recurrence reads pall and writes hsb directly (no bulk
                # staging copies).  The next stage's projection is interleaved
                # into the second half of the steps (one m-chunk per ~2 steps)
                # so its PE work hides under recurrence chain slack; its input
                # add fires mid-stage, safely after the lag-2 collective.
                nu = min(U, step_count)
                inter = INTERLEAVE_PROJ and STRAIGHT and s + 1 in group and nu >= 56
                if inter:
                    pall_n = papool.tile([128, NCH * SB], FP16, tag="pall")
                    xs_n, rv_n = stage_xin_dma(s + 1)
                    xin_n = None
                    ps_n = None
                if STRAIGHT:
                    pst = preload(pall3, 0)
                    for uu in range(nu):
                        pst = recurrence_step(
                            pall3, hsb3, uu, pst,
                            uu + 1 if uu + 1 < nu else None)
                        if inter:
                            # one 512-col projection MM per step (~270ns)
                            # fits the per-step PE gap during tanh/update
                            if uu == 14:
                                xin_n = stage_xin_add(xs_n, rv_n)
                            elif 16 <= uu < 16 + KC * NCH:
                                mi = uu - 16
                                m, k = mi // KC, mi % KC
                                if k == 0:
                                    ps_n = pg_pool.tile([128, 512], FP32,
                                                        tag="pg")
                                proj_mm(xin_n, pall_n, m, k, ps_n)
                    if inter:
                        palls[s + 1] = pall_n
                else:
                    with tc.For_i(0, step_count, nu) as i:
                        pst = preload(pall3, 0)
                        for uu in range(nu):
                            pst = recurrence_step(
                                pall3, hsb3, uu, pst,
                                uu + 1 if uu + 1 < nu else None)
                    if hasattr(i, "val"):
                        for r in i.val:
                            nc.free_register(r)

                # next stage's staging + projection (burst fallback when not
                # interleaved above)
                if s + 1 in group and not inter:
                    pall_n = papool.tile([128, NCH * SB], FP16, tag="pall")
                    xin_n = stage_xin(s + 1)
                    proj(xin_n, pall_n)
                    palls[s + 1] = pall_n

                if s < 2:
                    nc.vector.tensor_mul(hT[:], hT[:], hmask_s[:])
                if s == C - 1:
                    out0 = out_d[0].rearrange("(c p) b -> p c b", p=128)
                    nc.sync.dma_start(out0, h3)
                if s == NS - 1:
                    out1 = out_d[1].rearrange("(c p) b -> p c b", p=128)
                    nc.sync.dma_start(out1, h3)

                if s < C:
                    snd = snpool.tile([128, KC * SB], FP16, tag="snd")
                    nc.vector.tensor_mul(snd[:], hsb[:], smask_s[:])
                    for k in range(KC):
                        nc.gpsimd.dma_start(send_t[s % 2][1, k],
                                            snd[:, k * SB:(k + 1) * SB])
                    nc.gpsimd.collective_compute(
                        "ReduceScatter", ALU.add,
                        replica_groups=GROUPS,
                        ins=[send_t[s % 2][:].opt()],
                        outs=[recv_t[s % 2][:].opt()],
                    )

            if not last_ctx:
                nc.sync.dma_start(hstate_d[:, 0:KC * B], hT[:])


def prep_in_maps(x, Wr, br, Wu, bu, Wo, bo, fp8=True):
    x = np.asarray(x, np.float32)
    Wr, Wu, Wo = (np.asarray(a, np.float32) for a in (Wr, Wu, Wo))
    br, bu, bo = (np.asarray(a, np.float32) for a in (br, bu, bo))

    def quant(w):
        if fp8:
            return (w * SCL).astype(ml_dtypes.float8_e4m3)
        return w.astype(np.float16)

    # With fp8 recurrent weights (pre-scaled by SCL), the x-side weights and
    # biases are also pre-scaled by SCL so PSUM holds SCL*(pre-activation)
    # uniformly; the gate activations then apply scale=1/SCL.
    xscl = SCL if fp8 else 1.0

    layer_weights = []
    for l in range(L):
        w_ru_h = np.concatenate([Wr[l][:, D:], Wu[l][:, D:]], 0)   # (2H, H)
        w_ru_x = np.concatenate([Wr[l][:, :D], Wu[l][:, :D]], 0)
        whru = quant(np.ascontiguousarray(w_ru_h.T.reshape(KC, 128, 2 * H)))
        wxru = (w_ru_x.T.reshape(KC, 128, 2 * H) * xscl).astype(np.float16)
        who = quant(np.ascontiguousarray(Wo[l][:, D:].T.reshape(KC, 128, H)))
        wxo = (Wo[l][:, :D].T.reshape(KC, 128, H) * xscl).astype(np.float16)
        bb = np.concatenate([br[l], bu[l], bo[l]], 0)              # (3H,)
        bias = (np.ascontiguousarray(bb.reshape(NCH, 128).T) * xscl
                ).astype(np.float32)
        layer_weights.append(dict(whru=whru, wxru=wxru, who=who, wxo=wxo,
                                  bias=bias))

    in_maps = []
    for c in range(N_CORES):
        lane, layer = c % LANES, c // LANES
        m = dict(layer_weights[layer])
        if layer == 0:
            xs = x[lane * B:(lane + 1) * B]                        # (B, T, D)
            xt = np.ascontiguousarray(xs.transpose(2, 1, 0))       # (D, T, B)
            srcx = np.ascontiguousarray(
                xt.reshape(KC, 128, C, S * B).transpose(2, 0, 1, 3))
            m["srcx"] = srcx.astype(np.float16)
        else:
            m["srcx"] = np.zeros((C, KC, 128, SB), np.float16)
        mv = 1.0 if layer == 0 else 0.0
        m["smask"] = np.full((128, KC * SB), mv, np.float16)
        m["hmask"] = np.full((128, KC * B), mv, np.float16)
        in_maps.append(m)
    return in_maps


def gather_output(results):
    out = np.zeros((L, B_FULL, H), np.float32)
    for lane in range(LANES):
        o0 = np.asarray(results[lane]["out"], np.float32)          # (2, H, B)
        o1 = np.asarray(results[lane + LANES]["out"], np.float32)
        out[0, lane * B:(lane + 1) * B] = o0[0].T
        out[1, lane * B:(lane + 1) * B] = o1[1].T
    return out


def kernel2(x, Wr, br, Wu, bu, Wo, bo, fp8=True):
    nc = build_gru2(fp8=fp8)
    in_maps = prep_in_maps(x, Wr, br, Wu, bu, Wo, bo, fp8=fp8)
    res = run_bass_kernel_spmd(nc, in_maps, list(range(N_CORES)))
    return gather_output(res.results)




_NC_CACHE = {}

# Default precision for kernel(): fp8 recurrent weights (scaled by SCL with
# the activation-side 1/SCL compensation). Measured rel-err 1.30e-2 vs the
# fp32 reference (threshold 2e-2); fp16 fallback measured 1.25e-3.
FP8_DEFAULT = True


def _get_nc():
    key = (C, S, U, FP8_DEFAULT)
    if key not in _NC_CACHE:
        _NC_CACHE[key] = build_gru2(fp8=FP8_DEFAULT)
    return _NC_CACHE[key]


# ---- timing support (used by test.py only; harness never calls these) ----
TIMING_STEPS_PAIR = (16, S)
TIMING_SL_DELTA = NS * (S - 16)
REAL_STEP_LAYERS = NS * S


def build_for_timing(steps):
    return build_gru2(step_count=steps, fp8=FP8_DEFAULT,
                      unroll=min(U, steps))


def kernel(x, Wr, br, Wu, bu, Wo, bo):
    nc = _get_nc()
    in_maps = prep_in_maps(x, Wr, br, Wu, bu, Wo, bo, fp8=FP8_DEFAULT)
    res = run_bass_kernel_spmd(nc, in_maps, list(range(N_CORES)))
    return gather_output(res.results)



# revision 52
# speedup vs baseline: 1.4065x; 1.1768x over previous
"""Trainium2 Bass kernel for nn_ConvGRU: 2-layer GRU, B=32, T=512, D=H=512.

Layer-pipelined across 8 NeuronCores: cores 0-3 run layer 1 on batch
lanes of 8 sequences, cores 4-7 run layer 2 on the same lanes, halving
each core's sequential step count (576 pipeline step-slots with C=16
32-step chunks instead of 1024).  h1 chunks flow core i -> core i+4
via ReduceScatter(add) pair collectives: send slot0 = zeros, slot1 =
chunk * smask (smask = 1 only on layer-1 cores), so rank-0 cores
receive zeros and rank-1 cores receive exactly the h1 chunk.  Every
core computes proj_input = x_staged + recv -- layer-1 cores get x + 0,
layer-2 cores get 0 + h1 -- keeping the SPMD program fully symmetric
(no branches).

The 18 pipeline stages are split across 5 sequential TileContexts (4
stages each); h state and comm buffers persist across contexts in
internal DRAM.  Stages are emitted straight-line (no hardware loops,
no registers, no staging copies): the recurrence reads the projected
pregates (pall) and writes the h history (hsb) at static offsets.

Per-step recurrence schedule (the per-slot critical path):
- Recurrent weights are e4m3 fp8 scaled by 256 (FWL loads fp8 stationary
  4 elements/cycle vs 2 for fp16, halving the dominant LDWEIGHTS cost of
  48 weight tiles per step); the x-side pregates and biases are
  pre-scaled by 256 on the host and the gate activations apply
  scale=1/256, so PSUM arithmetic is exact in the scaled domain.
- One fused DVE copy preloads the next step's x-pregates into an
  alternate [128, 3G] PSUM tile (bufs=2 pool) while the current step
  computes; matmuls accumulate onto it with start=False.
- PE order per step: r-block (m-major, 16 MMs; sigmoid(r) fires as soon
  as it completes), u-block (runs under sigmoid/rh), o-block on r*h.
- Update algebra h' = (h - u*h) + u*o: u*h and (h - u*h) are computed
  during the o-matmuls, so only u*o and one add follow tanh; tanh and
  the final two ops are split into halves so the next step's first
  matmuls (which read only the first half of h, tracked by subtile
  deps) start before the full update retires.
- The per-step h snapshot into the staging buffer runs on the otherwise
  idle gpsimd engine.
"""

import os
import sys

import numpy as np
import ml_dtypes

sys.path.insert(0, "/opt/trn_rl_repo")
os.environ.setdefault("MYCRO_LOCAL_CACHE", "1")

import concourse.bass as bass  # noqa: E402
import concourse.tile as tile  # noqa: E402
from concourse import mybir  # noqa: E402
from concourse.bass import ds, make_scalar_value, RegisterHandles  # noqa: E402
from concourse.bass_utils import run_bass_kernel_spmd  # noqa: E402
from concourse.vector_clock import ScopedClock  # noqa: E402

FP16 = mybir.dt.float16
FP32 = mybir.dt.float32
FP8 = mybir.dt.float8e4
AF = mybir.ActivationFunctionType
ALU = mybir.AluOpType

N_CORES = 8
LANES = 4
B_FULL, T, D, H, L = 32, 512, 512, 512, 2
B = B_FULL // LANES            # 8 sequences per lane
KC = H // 128                  # 4 contraction chunks
MR = KC                        # psum out chunks per gate
NCH = 3 * MR                   # 12
C = int(os.environ.get("KERNEL_C", "16"))  # time chunks
S = T // C                     # steps per chunk
SB = S * B                     # columns per chunk
NS = C + 2                     # pipeline stages (lag 2)
SCL = 256.0                    # weight pre-scale (h/256 streamed)
U = 64                         # recurrence unroll (straight-line: >= S)
G = MR * B
GROUPS = [[i, i + LANES] for i in range(LANES)]
_DRAIN_CHUNK = 1
_GS = int(os.environ.get("KERNEL_GS", "18"))  # stages per TileContext
# (18 = the whole pipeline in ONE TileContext: no context boundaries, no
# drain/barrier pairs, no fp16 h round-trips through DRAM; measured best)
STAGE_GROUPS = [tuple(range(i, min(i + _GS, NS))) for i in range(0, NS, _GS)]
STRAIGHT = True                # emit stages without the 1-iteration For_i
INTERLEAVE_PROJ = False        # measured a wash vs the burst; keep burst


class _PatchedTileContext(tile.TileContext):
    """TileContext whose exit drain carries at most _DRAIN_CHUNK sem waits
    per drain instruction (walrus v3 codegen rejects multi-wait drains)."""

    def _drain_and_barrier(self, tick_clock, wait_clock):
        nc = self.nc
        drain_inst = nc.sync.drain()
        wait_clock.add_sem_waits(
            drain_inst.ins, ScopedClock({None: tick_clock.global_clock})
        )
        si = drain_inst.ins.sync_info
        waits = list(si.on_wait) if si is not None else []
        ups = list(si.on_update) if si is not None else []
        if len(waits) > _DRAIN_CHUNK:
            drain_inst.ins.sync_info = mybir.SyncInfo(
                on_wait=waits[:_DRAIN_CHUNK], on_update=[])
            rest = waits[_DRAIN_CHUNK:]
            for i in range(0, len(rest), _DRAIN_CHUNK):
                d2 = nc.sync.drain()
                d2.ins.sync_info = mybir.SyncInfo(
                    on_wait=rest[i:i + _DRAIN_CHUNK],
                    on_update=ups if i + _DRAIN_CHUNK >= len(rest) else [])
        nc.all_engine_barrier()
        popped = nc._tile_sem_poison_stack.pop()
        assert popped is self._sem_poison
        nc.clear_and_free_semaphores(list(self.sems.allocated().values()))
        nc.all_engine_barrier()


_MAX_WAITS = 1


def _split_sync_waits(nc, maxw=_MAX_WAITS):
    """walrus v2/v3 codegen rejects instructions carrying several sync
    waits ("Too many sync wait commands"); split them into preceding
    single-wait NoOps on the same engine."""
    n_new = 0
    for f in nc.m.functions:
        for bb in f.blocks:
            insts = list(bb.instructions)
            out = []
            changed = False
            for inst in insts:
                si = inst.sync_info
                waits = list(si.on_wait) if si is not None and si.on_wait else []
                if len(waits) > maxw:
                    ups = list(si.on_update) if si.on_update else []
                    k = len(waits)
                    for i in range(0, k - maxw, maxw):
                        nop = mybir.InstNoOp(
                            name=f"{inst.name}-wsplit{i}", engine=inst.engine,
                            sync_info=mybir.SyncInfo(
                                on_wait=waits[i:i + maxw], on_update=[]))
                        out.append(nop)
                        n_new += 1
                    inst.sync_info = mybir.SyncInfo(
                        on_wait=waits[k - maxw:], on_update=ups)
                    changed = True
                out.append(inst)
            if changed:
                bb.instructions = out
    return n_new


def build_gru2(step_count=None, fp8=True, unroll=None):
    if step_count is None:
        step_count = S
    WDT = FP8 if fp8 else FP16
    nc = bass.Bass(num_devices=N_CORES)

    whru_d = nc.declare_dram_parameter("whru", [KC, 128, 2 * H], WDT, isOutput=False)
    who_d = nc.declare_dram_parameter("who", [KC, 128, H], WDT, isOutput=False)
    wxru_d = nc.declare_dram_parameter("wxru", [KC, 128, 2 * H], FP16, isOutput=False)
    wxo_d = nc.declare_dram_parameter("wxo", [KC, 128, H], FP16, isOutput=False)
    bias_d = nc.declare_dram_parameter("bias", [128, NCH], FP32, isOutput=False)
    srcx_d = nc.declare_dram_parameter("srcx", [C, KC, 128, SB], FP16, isOutput=False)
    smask_d = nc.declare_dram_parameter("smask", [128, KC * SB], FP16, isOutput=False)
    hmask_d = nc.declare_dram_parameter("hmask", [128, KC * B], FP16, isOutput=False)
    out_d = nc.declare_dram_parameter("out", [2, H, B], FP16, isOutput=True)

    send_t = [nc.dram_tensor(f"sendbuf{p}", [2, KC, 128, SB], FP16, kind="Internal")
              for p in range(2)]
    recv_t = [nc.dram_tensor(f"recvbuf{p}", [KC, 128, SB], FP16, kind="Internal")
              for p in range(2)]
    hstate_d = nc.dram_tensor("hstate", [128, 2 * KC * B], FP16, kind="Internal")

    uu = U if unroll is None else unroll
    for gi, group in enumerate(STAGE_GROUPS):
        _build_group(nc, tuple(group), gi, step_count, WDT, uu,
                     whru_d, who_d, wxru_d, wxo_d, bias_d, srcx_d, smask_d,
                     hmask_d, out_d, send_t, recv_t, hstate_d)

    _split_sync_waits(nc)
    return nc


def _build_group(nc, group, gi, step_count, WDT, U,
                 whru_d, who_d, wxru_d, wxo_d, bias_d, srcx_d, smask_d,
                 hmask_d, out_d, send_t, recv_t, hstate_d):
    first_ctx = gi == 0
    last_ctx = gi == len(STAGE_GROUPS) - 1
    with _PatchedTileContext(nc) as tc:
        with (
            tc.tile_pool(name="weights", bufs=1) as wpool,
            tc.tile_pool(name="state", bufs=1) as stpool,
            tc.tile_pool(name="xs", bufs=2) as xspool,
            tc.tile_pool(name="rv", bufs=2) as rvpool,
            tc.tile_pool(name="xin", bufs=2) as xinpool,
            tc.tile_pool(name="pall", bufs=2) as papool,
            tc.tile_pool(name="hsb", bufs=2) as hbpool,
            tc.tile_pool(name="snd", bufs=2) as snpool,
            tc.tile_pool(name="psum_g", bufs=2, space="PSUM") as pg_pool,
            tc.tile_pool(name="psum_r", bufs=2, space="PSUM") as pr_pool,
        ):
            # ---- weights / constants ----
            whru_s = wpool.tile([128, KC * 2 * H], WDT, tag="whru")
            who_s = wpool.tile([128, KC * H], WDT, tag="who")
            wxru_s = wpool.tile([128, KC * 2 * H], FP16, tag="wxru")
            wxo_s = wpool.tile([128, KC * H], FP16, tag="wxo")
            bias_s = wpool.tile([128, NCH], FP32, tag="bias")
            for k in range(KC):
                nc.sync.dma_start(whru_s[:, k * 2 * H:(k + 1) * 2 * H], whru_d[k])
                nc.sync.dma_start(who_s[:, k * H:(k + 1) * H], who_d[k])
                nc.sync.dma_start(wxru_s[:, k * 2 * H:(k + 1) * 2 * H], wxru_d[k])
                nc.sync.dma_start(wxo_s[:, k * H:(k + 1) * H], wxo_d[k])
            nc.sync.dma_start(bias_s[:], bias_d[:])
            smask_s = hmask_s = None
            if any(s < C for s in group):
                smask_s = wpool.tile([128, KC * SB], FP16, tag="smask")
                nc.sync.dma_start(smask_s[:], smask_d[:])
            if any(s < 2 for s in group):
                hmask_s = wpool.tile([128, KC * B], FP16, tag="hmask")
                nc.sync.dma_start(hmask_s[:], hmask_d[:])

            # ---- state ----
            hT = stpool.tile([128, KC * B], FP16, tag="hT")
            if first_ctx:
                zt = stpool.tile([128, SB], FP16, tag="zt")
                nc.vector.memset(zt[:], 0.0)
                for p in range(2):
                    for k in range(KC):
                        nc.gpsimd.dma_start(send_t[p][0, k], zt[:])
                nc.vector.memset(hT[:], 0.0)
            else:
                nc.sync.dma_start(hT[:], hstate_d[:, 0:KC * B])
            h3 = hT[:].rearrange("p (c b) -> p c b", c=KC)

            gates = stpool.tile([128, 7 * G], FP16, tag="gates")
            rT = gates[:, 0:G]
            uT = gates[:, G:2 * G]
            oT = gates[:, 2 * G:3 * G]
            rhT = gates[:, 3 * G:4 * G]
            uhT = gates[:, 4 * G:5 * G]
            eT = gates[:, 5 * G:6 * G]
            fT = gates[:, 6 * G:7 * G]

            # ---- helpers ----
            def stage_xin_dma(s):
                """Issue the input DMAs for stage s; the combining add is
                deferred (see stage_xin)."""
                chunk = min(s, C - 1)
                xs = xspool.tile([128, KC * SB], FP16, tag="xs")
                for k in range(KC):
                    nc.sync.dma_start(xs[:, k * SB:(k + 1) * SB], srcx_d[chunk, k])
                if s < 2:
                    return xs, None
                rv = rvpool.tile([128, KC * SB], FP16, tag="rv")
                for k in range(KC):
                    nc.sync.dma_start(rv[:, k * SB:(k + 1) * SB], recv_t[s % 2][k])
                return xs, rv

            def stage_xin_add(xs, rv):
                if rv is None:
                    return xs
                xin = xinpool.tile([128, KC * SB], FP16, tag="xin")
                nc.vector.tensor_add(xin[:], xs[:], rv[:])
                return xin

            def stage_xin(s):
                xs, rv = stage_xin_dma(s)
                return stage_xin_add(xs, rv)

            def proj_mm(xin, pall, m, k, ps):
                """One projection matmul (m-chunk m, contraction chunk k);
                emits the bias/ACT when the accumulation group completes.
                SB<=512 assumed (single free-dim group)."""
                if m < 2 * MR:
                    lhsT = wxru_s[:, k * 2 * H + m * 128:
                                  k * 2 * H + (m + 1) * 128]
                else:
                    mo = m - 2 * MR
                    lhsT = wxo_s[:, k * H + mo * 128:
                                 k * H + (mo + 1) * 128]
                nc.tensor.matmul(ps[:, :SB], lhsT,
                                 xin[:, k * SB:(k + 1) * SB],
                                 start=(k == 0), stop=(k == KC - 1))
                if k == KC - 1:
                    nc.scalar.activation(
                        pall[:, m * SB:(m + 1) * SB], ps[:, :SB],
                        AF.Identity, bias=bias_s[:, m:m + 1])

            def proj_chunk(xin, pall, m):
                ps = pg_pool.tile([128, 512], FP32, tag="pg")
                for k in range(KC):
                    proj_mm(xin, pall, m, k, ps)

            def proj(xin, pall):
                for m in range(NCH):
                    proj_chunk(xin, pall, m)

            ascl = 1.0 / SCL if WDT == FP8 else 1.0
            HB = KC * B // 2          # half of the h free-dim (k-chunks 0-1)

            def preload(stage3, uu):
                """One fused DVE copy of step uu's x-pregates into a fresh
                [128, 3G] PSUM tile (bufs=2 pool => runs under step uu-1)."""
                pst = pr_pool.tile([128, NCH * B], FP32, tag="pst")
                pst3 = pst[:].rearrange("p (c b) -> p c b", c=NCH)
                nc.vector.tensor_copy(pst3, stage3[:, :, uu * B:(uu + 1) * B])
                return pst

            def recurrence_step(stage3, hstage3, uu, pst, pst_next_uu):
                s0 = uu * B
                psr = pst[:, 0:G]
                psu = pst[:, G:2 * G]
                pso = pst[:, 2 * G:3 * G]
                # m-major r block first so sigmoid(r) fires after 16 MMs;
                # u block follows (its sigmoid is only needed late).  The
                # first MM of each m-block reads h[k0] only, so the next
                # step still starts on the early-retired first h half
                # (subtile deps).
                for gm in range(2 * MR):
                    for k in range(KC):
                        nc.tensor.matmul(
                            pst[:, gm * B:(gm + 1) * B],
                            whru_s[:, k * 2 * H + gm * 128:
                                   k * 2 * H + (gm + 1) * 128],
                            hT[:, k * B:(k + 1) * B],
                            start=False, stop=(k == KC - 1),
                            skip_group_check=True)
                    if gm == MR - 1:
                        nc.scalar.activation(rT, psr, AF.Sigmoid, scale=ascl)
                nc.scalar.activation(uT, psu, AF.Sigmoid, scale=ascl)
                nc.vector.tensor_mul(rhT, rT, hT[:])
                nc.vector.tensor_mul(uhT, uT, hT[:])
                nc.vector.tensor_sub(eT, hT[:], uhT)   # e = (1-u)*h
                for m in range(MR):
                    for k in range(KC):
                        nc.tensor.matmul(
                            pso[:, m * B:(m + 1) * B],
                            who_s[:, k * H + m * 128:k * H + (m + 1) * 128],
                            rhT[:, k * B:(k + 1) * B],
                            start=False, stop=(k == KC - 1),
                            skip_group_check=True)
                # preload the next step's pregates while o/tanh run
                pst_next = None
                if pst_next_uu is not None:
                    pst_next = preload(stage3, pst_next_uu)
                # tanh + h-update in halves: h = e + u*o, first half retires
                # early so the next step's k=0 sweep can begin
                for hh in (0, 1):
                    lo, hi = hh * HB, (hh + 1) * HB
                    nc.scalar.activation(oT[:, lo:hi], pso[:, lo:hi],
                                         AF.Tanh, scale=ascl)
                    nc.vector.tensor_mul(fT[:, lo:hi], uT[:, lo:hi],
                                         oT[:, lo:hi])
                    nc.vector.tensor_add(hT[:, lo:hi], eT[:, lo:hi],
                                         fT[:, lo:hi])
                nc.gpsimd.tensor_copy(hstage3[:, :, s0:s0 + B], h3)
                return pst_next

            # ---- group prologue: first stage's xin + proj ----
            palls = {}
            xin0 = stage_xin(group[0])
            pall0 = papool.tile([128, NCH * SB], FP16, tag="pall")
            proj(xin0, pall0)
            palls[group[0]] = pall0

            # ---- stages ----
            for s in group:
                pall = palls.pop(s)
                pall3 = pall[:].rearrange("p (c t) -> p c t", c=NCH)
                hsb = hbpool.tile([128, KC * SB], FP16, tag="hsb")
                hsb3 = hsb[:].rearrange("p (c t) -> p c t", c=KC)

                # single-iteration (or no) hardware loop: all offsets static,
                # the recurrence reads pall and writes hsb directly (no bulk
                # staging copies).  The next stage's projection is interleaved
                # into the second half of the steps (one m-chunk per ~2 steps)
                # so its PE work hides under recurrence chain slack; its input
                # add fires mid-stage, safely after the lag-2 collective.
                nu = min(U, step_count)
                inter = INTERLEAVE_PROJ and STRAIGHT and s + 1 in group and nu >= 56
                if inter:
                    pall_n = papool.tile([128, NCH * SB], FP16, tag="pall")
                    xs_n, rv_n = stage_xin_dma(s + 1)
                    xin_n = None
                    ps_n = None
                if STRAIGHT:
                    pst = preload(pall3, 0)
                    for uu in range(nu):
                        pst = recurrence_step(
                            pall3, hsb3, uu, pst,
                            uu + 1 if uu + 1 < nu else None)
                        if inter:
                            # one 512-col projection MM per step (~270ns)
                            # fits the per-step PE gap during tanh/update
                            if uu == 14:
                                xin_n = stage_xin_add(xs_n, rv_n)
                            elif 16 <= uu < 16 + KC * NCH:
                                mi = uu - 16
                                m, k = mi // KC, mi % KC
                                if k == 0:
                                    ps_n = pg_pool.tile([128, 512], FP32,
                                                        tag="pg")
                                proj_mm(xin_n, pall_n, m, k, ps_n)
                    if inter:
                        palls[s + 1] = pall_n
                else:
                    with tc.For_i(0, step_count, nu) as i:
                        pst = preload(pall3, 0)
                        for uu in range(nu):
                            pst = recurrence_step(
                                pall3, hsb3, uu, pst,
                                uu + 1 if uu + 1 < nu else None)
                    if hasattr(i, "val"):
                        for r in i.val:
                            nc.free_register(r)

                # next stage's staging + projection (burst fallback when not
                # interleaved above)
                if s + 1 in group and not inter:
                    pall_n = papool.tile([128, NCH * SB], FP16, tag="pall")
                    xin_n = stage_xin(s + 1)
                    proj(xin_n, pall_n)
                    palls[s + 1] = pall_n

                if s < 2:
                    nc.vector.tensor_mul(hT[:], hT[:], hmask_s[:])
                if s == C - 1:
                    out0 = out_d[0].rearrange("(c p) b -> p c b", p=128)
                    nc.sync.dma_start(out0, h3)
                if s == NS - 1:
                    out1 = out_d[1].rearrange("(c p) b -> p c b", p=128)
                    nc.sync.dma_start(out1, h3)

                if s < C:
                    snd = snpool.tile([128, KC * SB], FP16, tag="snd")
                    nc.vector.tensor_mul(snd[:], hsb[:], smask_s[:])
                    for k in range(KC):
                        nc.gpsimd.dma_start(send_t[s % 2][1, k],
                                            snd[:, k * SB:(k + 1) * SB])
                    nc.gpsimd.collective_compute(
                        "ReduceScatter", ALU.add,
                        replica_groups=GROUPS,
                        ins=[send_t[s % 2][:].opt()],
                        outs=[recv_t[s % 2][:].opt()],
                    )

            if not last_ctx:
                nc.sync.dma_start(hstate_d[:, 0:KC * B], hT[:])


def prep_in_maps(x, Wr, br, Wu, bu, Wo, bo, fp8=True):
    x = np.asarray(x, np.float32)
    Wr, Wu, Wo = (np.asarray(a, np.float32) for a in (Wr, Wu, Wo))
    br, bu, bo = (np.asarray(a, np.float32) for a in (br, bu, bo))

    def quant(w):
        if fp8:
            return (w * SCL).astype(ml_dtypes.float8_e4m3)
        return w.astype(np.float16)

    # With fp8 recurrent weights (pre-scaled by SCL), the x-side weights and
    # biases are also pre-scaled by SCL so PSUM holds SCL*(pre-activation)
    # uniformly; the gate activations then apply scale=1/SCL.
    xscl = SCL if fp8 else 1.0

    layer_weights = []
    for l in range(L):
        w_ru_h = np.concatenate([Wr[l][:, D:], Wu[l][:, D:]], 0)   # (2H, H)
        w_ru_x = np.concatenate([Wr[l][:, :D], Wu[l][:, :D]], 0)
        whru = quant(np.ascontiguousarray(w_ru_h.T.reshape(KC, 128, 2 * H)))
        wxru = (w_ru_x.T.reshape(KC, 128, 2 * H) * xscl).astype(np.float16)
        who = quant(np.ascontiguousarray(Wo[l][:, D:].T.reshape(KC, 128, H)))
        wxo = (Wo[l][:, :D].T.reshape(KC, 128, H) * xscl).astype(np.float16)
        bb = np.concatenate([br[l], bu[l], bo[l]], 0)              # (3H,)
        bias = (np.ascontiguousarray(bb.reshape(NCH, 128).T) * xscl
                ).astype(np.float32)
        layer_weights.append(dict(whru=whru, wxru=wxru, who=who, wxo=wxo,
                                  bias=bias))

    in_maps = []
    for c in range(N_CORES):
        lane, layer = c % LANES, c // LANES
        m = dict(layer_weights[layer])
        if layer == 0:
            xs = x[lane * B:(lane + 1) * B]                        # (B, T, D)
            xt = np.ascontiguousarray(xs.transpose(2, 1, 0))       # (D, T, B)
            srcx = np.ascontiguousarray(
                xt.reshape(KC, 128, C, S * B).transpose(2, 0, 1, 3))
            m["srcx"] = srcx.astype(np.float16)
        else:
            m["srcx"] = np.zeros((C, KC, 128, SB), np.float16)
        mv = 1.0 if layer == 0 else 0.0
        m["smask"] = np.full((128, KC * SB), mv, np.float16)
        m["hmask"] = np.full((128, KC * B), mv, np.float16)
        in_maps.append(m)
    return in_maps


def gather_output(results):
    out = np.zeros((L, B_FULL, H), np.float32)
    for lane in range(LANES):
        o0 = np.asarray(results[lane]["out"], np.float32)          # (2, H, B)
        o1 = np.asarray(results[lane + LANES]["out"], np.float32)
        out[0, lane * B:(lane + 1) * B] = o0[0].T
        out[1, lane * B:(lane + 1) * B] = o1[1].T
    return out


def kernel2(x, Wr, br, Wu, bu, Wo, bo, fp8=True):
    nc = build_gru2(fp8=fp8)
    in_maps = prep_in_maps(x, Wr, br, Wu, bu, Wo, bo, fp8=fp8)
    res = run_bass_kernel_spmd(nc, in_maps, list(range(N_CORES)))
    return gather_output(res.results)




_NC_CACHE = {}

# Default precision for kernel(): fp8 recurrent weights (scaled by SCL with
# the activation-side 1/SCL compensation). Measured rel-err 1.30e-2 vs the
# fp32 reference (threshold 2e-2); fp16 fallback measured 1.25e-3.
FP8_DEFAULT = True


def _get_nc():
    key = (C, S, U, FP8_DEFAULT)
    if key not in _NC_CACHE:
        _NC_CACHE[key] = build_gru2(fp8=FP8_DEFAULT)
    return _NC_CACHE[key]


# ---- timing support (used by test.py only; harness never calls these) ----
TIMING_STEPS_PAIR = (16, S)
TIMING_SL_DELTA = NS * (S - 16)
REAL_STEP_LAYERS = NS * S


def build_for_timing(steps):
    return build_gru2(step_count=steps, fp8=FP8_DEFAULT,
                      unroll=min(U, steps))


def kernel(x, Wr, br, Wu, bu, Wo, bo):
    nc = _get_nc()
    in_maps = prep_in_maps(x, Wr, br, Wu, bu, Wo, bo, fp8=FP8_DEFAULT)
    res = run_bass_kernel_spmd(nc, in_maps, list(range(N_CORES)))
    return gather_output(res.results)

